# revision 23
# baseline (speedup 1.0000x reference)
"""Trainium2 Bass kernel for ConvReverseDataNet (USRNet-style FFT data step).

128-grid reformulation (per (b,c) plane, sf=2), validated in fp64/bf16 numpy:
  g   = DFT2_128(x)
  s_pq[a,b] = rolled_psf256[2a+p, 2b+q]   (parity subkernels of the 25x25 psf)
  Khat_pq = DFT2_128(s_pq) * e^{-2pi i(u p + v q)/128}  (twiddles folded into
            the small-DFT constants via positions (j+par)/2)
  W'  = sum_pq |Khat_pq|^2 ;  Y0' = sum_pq Khat_pq      (both on 128 grid)
  wt  = (1 - Y0') / (W' + be) ;  v = g * wt
  out[2m+p, 2n+q] = real(IDFT2_128(conj(Khat_pq) * v))[m,n] + x[m,n]

mm(stat, mov) = stat.T @ mov (contract over partition dim). Complex arrays
X stored as adjacent [Xr | Xs] blocks. Big-DFT matmuls run in float32r
(1 cycle/row at N>=256); the M/ifft stages run in bf16 (validated 3.2e-3).

Sharding: 256 (b,c) planes over 8 cores; core ci gets channels ci*8..ci*8+7.
Output is parity-planar [plane, 2p+q, m, n]; host interleaves.
"""

import functools
import sys

import ml_dtypes
import numpy as np

if "/opt/trn_rl_repo" not in sys.path:
    sys.path.insert(0, "/opt/trn_rl_repo")

from concourse import bacc, bass, mybir, tile  # noqa: E402
from concourse.bass_utils import run_bass_kernel_spmd  # noqa: E402

F32 = mybir.dt.float32
F32R = mybir.dt.float32r
BF16 = mybir.dt.bfloat16
MULT = mybir.AluOpType.mult
ADD = mybir.AluOpType.add
ACT_COPY = mybir.ActivationFunctionType.Copy

N_CORES = 8
NPL = 32  # planes per core
KS = 25


def _host_consts():
    t = np.arange(KS)
    j = (t - 12) % 256            # position in rolled 256 grid
    par = t % 2                   # parity (row p / col q)
    bp = (j + par) // 2           # twiddle-folded 128-grid position

    u = np.arange(128)
    ang = 2 * np.pi * np.outer(bp, u) / 128  # [25,128]
    COSg = np.cos(ang).astype(np.float32)
    SINg = np.sin(ang).astype(np.float32)
    sel = [(par == 0).astype(np.float32)[:, None], (par == 1).astype(np.float32)[:, None]]

    c = {}
    c["WMOV"] = np.concatenate(
        [COSg * sel[0], COSg * sel[1], -SINg * sel[0], -SINg * sel[1]], axis=1
    )  # [25,512] -> A blocks [Ar0|Ar1|As0|As1]
    for p in (0, 1):
        c[f"ZC{p}"] = COSg * sel[p]
        c[f"ZS{p}"] = -SINg * sel[p]
        c[f"ZSn{p}"] = SINg * sel[p]

    th = 2 * np.pi * np.outer(u, u) / 128
    C1 = np.cos(th).astype(np.float32)
    S1 = np.sin(th).astype(np.float32)
    c["CS"] = np.concatenate([C1, S1], 1)
    c["CSn"] = np.concatenate([C1, -S1], 1)
    c["SnCn"] = np.concatenate([-S1, -C1], 1)
    c["ICSf"] = np.concatenate([C1, S1], 1)
    c["ISnCf"] = np.concatenate([-S1, C1], 1)
    c["Cscf"] = C1 / 16384.0
    c["Sscnf"] = -S1 / 16384.0
    return {n: np.ascontiguousarray(a, dtype=np.float32) for n, a in c.items()}


CONST_SHAPES = {
    "WMOV": [KS, 512],
    "ZC0": [KS, 128], "ZC1": [KS, 128],
    "ZS0": [KS, 128], "ZS1": [KS, 128],
    "ZSn0": [KS, 128], "ZSn1": [KS, 128],
    "CS": [128, 256], "CSn": [128, 256], "SnCn": [128, 256],
    "ICSf": [128, 256], "ISnCf": [128, 256],
    "Cscf": [128, 128], "Sscnf": [128, 128],
}


def build_nc(n_planes=NPL):
    nc = bacc.Bacc("TRN2", target_bir_lowering=False, debug=False, num_devices=N_CORES)

    xs_t = nc.dram_tensor("xs", [n_planes, 128, 128], F32, kind="ExternalInput")
    xsb_t = nc.dram_tensor("xsb", [n_planes, 128, 128], BF16, kind="ExternalInput")
    kt_t = nc.dram_tensor("kt", [n_planes, KS, KS], BF16, kind="ExternalInput")
    be_t = nc.dram_tensor("besb", [128, n_planes], F32, kind="ExternalInput")
    const_t = {n: nc.dram_tensor(n, s, (F32 if n.endswith("f") else BF16), kind="ExternalInput") for n, s in CONST_SHAPES.items()}
    out_t = nc.dram_tensor("out", [n_planes, 4, 128, 128], F32, kind="ExternalOutput")

    X = mybir.AxisListType.X

    with tile.TileContext(nc) as tc:
        with (
            tc.tile_pool(name="cpool", bufs=1) as cpool,
            tc.tile_pool(name="small", bufs=6) as small,
            tc.tile_pool(name="med", bufs=4) as med,
            tc.tile_pool(name="psum", bufs=1, space="PSUM") as pp,
        ):
            cs = {}
            for n, s in CONST_SHAPES.items():
                cs[n] = cpool.tile(s, (F32 if n.endswith("f") else BF16), tag=n, name=f"c_{n}")
                nc.sync.dma_start(cs[n][:], const_t[n][:])
            besb = cpool.tile([128, n_planes], F32, tag="besb")
            nc.sync.dma_start(besb[:], be_t[:])

            # one-time bf16 conversions for the ifft constants
            ICS = cpool.tile([128, 256], BF16, tag="ICS")
            ISnC = cpool.tile([128, 256], BF16, tag="ISnC")
            Csc = cpool.tile([128, 128], BF16, tag="Csc")
            Sscn = cpool.tile([128, 128], BF16, tag="Sscn")
            nc.scalar.copy(ICS[:], cs["ICSf"][:])
            nc.scalar.copy(ISnC[:], cs["ISnCf"][:])
            nc.scalar.copy(Csc[:], cs["Cscf"][:])
            nc.scalar.copy(Sscn[:], cs["Sscnf"][:])

            for i in range(n_planes):
                # ---- A = kt^T-transform of psf columns ----
                kt_sb = small.tile([KS, KS], BF16, tag="kt_sb")
                nc.sync.dma_start(kt_sb[:], kt_t[i])
                Aps = pp.tile([KS, 512], F32, tag="Aps")
                nc.tensor.matmul(Aps[:], kt_sb[:], cs["WMOV"][:], start=True, stop=True)
                AX = med.tile([KS, 512], BF16, tag="AX")
                nc.scalar.copy(AX[:], Aps[:])

                # ---- Khat per row-parity p: psum [Kr_p0|Kr_p1|Ks_p0|Ks_p1] ----
                Kps = []
                for p in (0, 1):
                    kp = pp.tile([128, 512], F32, tag="big512", bufs=2)
                    ZC, ZS, ZSn = cs[f"ZC{p}"], cs[f"ZS{p}"], cs[f"ZSn{p}"]
                    nc.tensor.matmul(kp[:, 0:256], ZC[:], AX[:, 0:256], start=True, stop=False)
                    nc.tensor.matmul(kp[:, 0:256], ZSn[:], AX[:, 256:512], start=False, stop=True)
                    nc.tensor.matmul(kp[:, 256:512], ZC[:], AX[:, 256:512], start=True, stop=False)
                    nc.tensor.matmul(kp[:, 256:512], ZS[:], AX[:, 0:256], start=False, stop=True)
                    Kps.append(kp)

                # ---- T (bf16 Khat, both parities) ----
                Tall = med.tile([128, 1024], BF16, tag="Tall")
                nc.scalar.copy(Tall[:, 0:512], Kps[0][:])
                nc.scalar.copy(Tall[:, 512:1024], Kps[1][:])
                v8 = Tall[:].rearrange("p (P h q f) -> p P h q f", P=2, h=2, q=2)

                # ---- Y0' = sum_pq Khat_pq via gpsimd folds ----
                y0a = small.tile([128, 256], F32, tag="y0a")
                y0b = small.tile([128, 256], F32, tag="y0b")
                Y0f = small.tile([128, 256], F32, tag="Y0f")  # [Y0r | Y0s]
                hf = lambda ap: ap.rearrange("p (h f) -> p h f", h=2)  # noqa: E731
                nc.gpsimd.tensor_add(hf(y0a[:]), v8[:, 0, :, 0, :], v8[:, 0, :, 1, :])
                nc.gpsimd.tensor_add(hf(y0b[:]), v8[:, 1, :, 0, :], v8[:, 1, :, 1, :])
                nc.gpsimd.tensor_add(Y0f[:], y0a[:], y0b[:])
                sq0 = med.tile([128, 512], F32, tag="sq0")
                sq1 = med.tile([128, 512], F32, tag="sq1")
                nc.scalar.square(sq0[:], Kps[0][:])
                nc.scalar.square(sq1[:], Kps[1][:])
                sqs = med.tile([128, 512], F32, tag="sqs")
                nc.gpsimd.tensor_add(sqs[:], sq0[:], sq1[:])
                sqh = small.tile([128, 256], F32, tag="sqh")
                nc.vector.tensor_add(sqh[:], sqs[:, 0:256], sqs[:, 256:512])
                Wt = small.tile([128, 128], F32, tag="Wt")
                nc.vector.tensor_add(Wt[:], sqh[:, 0:128], sqh[:, 128:256])

                # ---- wt = (1 - Y0')/(W' + be) ----
                den = small.tile([128, 128], F32, tag="den")
                nc.vector.tensor_scalar_add(den[:], Wt[:], besb[:, i:i + 1])
                dinv = small.tile([128, 128], F32, tag="dinv")
                nc.vector.reciprocal_approx_fast(dinv[:], den[:])
                num = small.tile([128, 128], F32, tag="num")
                nc.scalar.activation(num[:], Y0f[:, 0:128], ACT_COPY, bias=1.0, scale=-1.0)
                wtile = small.tile([128, 256], BF16, tag="wtile")  # [wr | ws]
                nc.vector.tensor_mul(wtile[:, 0:128], num[:], dinv[:])
                nc.vector.scalar_tensor_tensor(wtile[:, 128:256], Y0f[:, 128:256], -1.0, dinv[:], MULT, MULT)

                # ---- g = DFT2_128(x) ----
                x_sb = small.tile([128, 128], F32, tag="x_sb")
                nc.sync.dma_start(x_sb[:], xs_t[i])
                x_sbb = small.tile([128, 128], BF16, tag="x_sbb")
                nc.sync.dma_start(x_sbb[:], xsb_t[i])
                Pps = pp.tile([128, 256], F32, tag="fft256", bufs=2)
                nc.tensor.matmul(Pps[:], x_sbb[:], cs["CS"][:], start=True, stop=True)
                Pcp = small.tile([128, 256], BF16, tag="Pcp")
                nc.scalar.copy(Pcp[:], Pps[:])
                gps = pp.tile([128, 256], F32, tag="fft256", bufs=2)
                nc.tensor.matmul(gps[:], Pcp[:, 0:128], cs["CSn"][:], start=True, stop=False)
                nc.tensor.matmul(gps[:], Pcp[:, 128:256], cs["SnCn"][:], start=False, stop=True)
                gb = small.tile([128, 256], BF16, tag="gb")
                nc.scalar.copy(gb[:], gps[:])

                # ---- v = g * wt (bf16): t_a=[gr wr|gr ws], t_b=[gs wr|gs ws] ----
                b2 = lambda ap: ap.rearrange("p (b f) -> p b f", b=2)  # noqa: E731
                t_a = small.tile([128, 256], BF16, tag="t_a")
                t_b = small.tile([128, 256], BF16, tag="t_b")
                gr2 = gb[:, 0:128].unsqueeze(1).broadcast_to([128, 2, 128])
                gs2 = gb[:, 128:256].unsqueeze(1).broadcast_to([128, 2, 128])
                nc.vector.tensor_mul(b2(t_a[:]), gr2, b2(wtile[:]))
                nc.vector.tensor_mul(b2(t_b[:]), gs2, b2(wtile[:]))
                vr = small.tile([128, 128], BF16, tag="vr")
                vs = small.tile([128, 128], BF16, tag="vs")
                nc.vector.tensor_sub(vr[:], t_a[:, 0:128], t_b[:, 128:256])
                nc.vector.tensor_add(vs[:], t_a[:, 128:256], t_b[:, 0:128])
                vr4 = vr[:].unsqueeze(1).unsqueeze(1).broadcast_to([128, 2, 2, 128])
                vs4 = vs[:].unsqueeze(1).unsqueeze(1).broadcast_to([128, 2, 2, 128])

                # ---- M_pq = conj(Khat_pq) * v (wide ops) ----
                KRv = v8[:, :, 0, :, :]   # [128, P, q, 128] real parts
                KSv = v8[:, :, 1, :, :]   # imag parts
                b4 = lambda ap: ap.rearrange("p (P q f) -> p P q f", P=2, q=2)  # noqa: E731
                m1 = med.tile([128, 512], BF16, tag="m1")
                m2 = med.tile([128, 512], BF16, tag="m2")
                MR = med.tile([128, 512], BF16, tag="MR")
                nc.vector.tensor_mul(b4(m1[:]), KRv, vr4)
                nc.vector.tensor_mul(b4(m2[:]), KSv, vs4)
                nc.vector.tensor_add(MR[:], m1[:], m2[:])
                m3 = med.tile([128, 512], BF16, tag="m3")
                m4 = med.tile([128, 512], BF16, tag="m4")
                MS = med.tile([128, 512], BF16, tag="MS")
                nc.vector.tensor_mul(b4(m3[:]), KRv, vs4)
                nc.vector.tensor_mul(b4(m4[:]), KSv, vr4)
                nc.vector.tensor_sub(MS[:], m3[:], m4[:])

                # ---- ifft + x ; store ----
                out_sb = med.tile([128, 512], F32, tag="out_sb")
                R2ps = pp.tile([128, 512], F32, tag="R2ps", bufs=2)
                for p in (0, 1):
                    R1ps = pp.tile([128, 512], F32, tag="R1ps")
                    for q in (0, 1):
                        blk = 2 * p + q
                        rsl = slice(q * 256, (q + 1) * 256)
                        bsl = slice(blk * 128, (blk + 1) * 128)
                        nc.tensor.matmul(R1ps[:, rsl], MR[:, bsl], ICS[:], start=True, stop=False)
                        nc.tensor.matmul(R1ps[:, rsl], MS[:, bsl], ISnC[:], start=False, stop=True)
                        R1cp = small.tile([128, 256], BF16, tag="R1cp")
                        nc.scalar.copy(R1cp[:], R1ps[:, rsl])
                        nc.tensor.matmul(R2ps[:, bsl], R1cp[:, 0:128], Csc[:], start=True, stop=False)
                        nc.tensor.matmul(R2ps[:, bsl], R1cp[:, 128:256], Sscn[:], start=False, stop=True)
                        nc.vector.tensor_add(out_sb[:, bsl], R2ps[:, bsl], x_sb[:])

                nc.sync.dma_start(
                    out_t[i].rearrange("pq m f -> m pq f"),
                    out_sb[:].rearrange("p (pq f) -> p pq f", pq=4),
                )

    nc.compile()
    return nc


@functools.lru_cache(maxsize=2)
def _built(n_planes=NPL):
    return build_nc(n_planes)


def make_in_maps(x, k, alpha, n_planes=NPL, n_cores=N_CORES):
    consts = _host_consts()
    alpha_c = alpha.reshape(-1).astype(np.float64)  # [64]
    be = (1.0 / (1.0 + np.exp(-(alpha_c - 9.0))) + 1e-3).astype(np.float32)
    cpc = n_planes // 4  # channels per core
    in_maps = []
    for ci in range(n_cores):
        chs = slice(ci * cpc, (ci + 1) * cpc)
        xs = np.ascontiguousarray(x[:, chs].transpose(1, 0, 2, 3).reshape(n_planes, 128, 128))
        kt = np.ascontiguousarray(k[:, chs].transpose(1, 0, 3, 2).reshape(n_planes, KS, KS))
        be_pl = np.repeat(be[chs], 4)  # plane order: (c_loc, b)
        besb = np.broadcast_to(be_pl, (128, n_planes)).astype(np.float32).copy()
        m = {"xs": xs, "xsb": xs.astype(ml_dtypes.bfloat16),
             "kt": kt.astype(ml_dtypes.bfloat16), "besb": besb}
        m.update({n: (a if n.endswith("f") else a.astype(ml_dtypes.bfloat16))
                  for n, a in consts.items()})
        in_maps.append(m)
    return in_maps


def _assemble(res, n_planes=NPL, n_cores=N_CORES):
    out = np.empty((4, 64, 256, 256), np.float32)
    cpc = n_planes // 4
    for ci in range(n_cores):
        o = res.results[ci]["out"].reshape(cpc, 4, 2, 2, 128, 128)
        # [c_loc, b, p, q, m, n] -> [b, c_loc, m, p, n, q] -> [b, c_loc, 256, 256]
        o = o.transpose(1, 0, 4, 2, 5, 3).reshape(4, cpc, 256, 256)
        out[:, ci * cpc:(ci + 1) * cpc] = o
    return out


def kernel(x, k, alpha, sf=2, **_ignored):
    x = np.asarray(x, dtype=np.float32)
    k = np.asarray(k, dtype=np.float32)
    alpha = np.asarray(alpha, dtype=np.float32)
    assert int(sf) == 2 and x.shape == (4, 64, 128, 128) and k.shape == (4, 64, KS, KS)

    nc = _built(NPL)
    in_maps = make_in_maps(x, k, alpha)
    res = run_bass_kernel_spmd(nc, in_maps, core_ids=list(range(N_CORES)))
    return _assemble(res)


if __name__ == "__main__":
    rng = np.random.default_rng(0)
    x = rng.standard_normal((4, 64, 128, 128), dtype=np.float32)
    k = rng.random((4, 64, KS, KS), dtype=np.float32)
    alpha = np.zeros((1, 64, 1, 1), np.float32)
    out = kernel(x, k, alpha, 2)
    print("out", out.shape, out.dtype, float(np.abs(out).max()))
